# revision 15
# baseline (speedup 1.0000x reference)
"""DAFusion fused kernel for Trainium2, 8-core data-parallel over batch.

Layout: activations are feature-major ([D, tokens]) in SBUF so every
projection is out = W.T @ xT with W in natural [in, out] layout as the
stationary operand. Token-major DRAM inputs are transposed on-chip via PE
transpose-mode. Matmuls run as float32r (FP22-truncated fp32, full rate at
N>=256). Attention dots are computed pre-transposed ([j, i]) so softmax
denominators come from ones-matmuls and P.T feeds the PV matmul directly;
exp skips max-subtraction (|scaled dots| stays small for this model).
The triple contraction einsum('bvd,bhvq,bqd->bhd').sum(h) is computed as
sum_v ga[v,d] * (S @ la)[v,d] with S = sum_h scores_h accumulated in PSUM.
"""

import sys

sys.path.insert(0, "/opt/trn_rl_repo")

from contextlib import ExitStack

import numpy as np

import concourse.bacc as bacc
import concourse.mybir as mybir
import concourse.tile as tile
N_CORES = 8
B, V, Q = 32, 512, 512
GD, LD, HID, HOUT = 512, 256, 128, 4
DIM = 384
TH, THD = 3, 32
DH = 96
BPC = B // N_CORES
F32 = mybir.dt.float32
F32R = mybir.dt.float32r
AF = mybir.ActivationFunctionType
ALU = mybir.AluOpType
AX = mybir.AxisListType
EPS = 1e-5


def build_bass():
    nc = bacc.Bacc(
        "TRN2", target_bir_lowering=False, debug=False, num_devices=N_CORES
    )

    din = {}
    din["g"] = nc.dram_tensor("g", [BPC, V, GD], F32, kind="ExternalInput")
    din["l"] = nc.dram_tensor("l", [BPC, Q, LD], F32, kind="ExternalInput")
    for name, shape in [
        ("Wg", [GD, DIM]), ("bg", [DIM]), ("Wl", [LD, DIM]), ("bl", [DIM]),
        ("ln1_g", [DIM]), ("ln1_b", [DIM]),
        ("Wqkv", [DIM, TH * THD * 3]),
        ("Wo", [TH * THD, DIM]), ("bo", [DIM]),
        ("ln2_g", [DIM]), ("ln2_b", [DIM]),
        ("W1", [DIM, 128]), ("b1", [128]), ("W2", [128, DIM]), ("b2", [DIM]),
        ("Wq", [DIM, HOUT * DH]), ("Wk", [DIM, HOUT * DH]),
        ("att_bias", [1, HOUT, 1, 1]),
        ("bn_g", [HID]), ("bn_b", [HID]), ("bn_mean", [HID]), ("bn_var", [HID]),
        ("Wf", [HID, 1]), ("bf", [1]),
    ]:
        din[name] = nc.dram_tensor(name, shape, F32, kind="ExternalInput")

    for name, shape in [("c_ident", [128, 128]), ("c_ones", [128, 128]),
                        ("c_onesinv", [128, 128]), ("c_eps", [128, 1])]:
        din[name] = nc.dram_tensor(name, shape, F32, kind="ExternalInput")

    scores_out = nc.dram_tensor(
        "scores", [BPC, HOUT, V, Q], F32, kind="ExternalOutput"
    )
    out_out = nc.dram_tensor("out", [BPC, 1], F32, kind="ExternalOutput")

    with tile.TileContext(nc) as tc, ExitStack() as ctx:
        build_kernel(ctx, tc, nc, din, scores_out, out_out)

    nc.compile()
    return nc


def build_kernel(ctx, tc, nc, din, scores_out, out_out):
    sg = ctx.enter_context(tc.tile_pool(name="singles", bufs=1))
    sb_in = ctx.enter_context(tc.tile_pool(name="sb_in", bufs=1))
    sb_act = ctx.enter_context(tc.tile_pool(name="sb_act", bufs=1))
    sb_tr = ctx.enter_context(tc.tile_pool(name="sb_tr", bufs=1))
    sb_stg = ctx.enter_context(tc.tile_pool(name="sb_stg", bufs=4))
    sb_small = ctx.enter_context(tc.tile_pool(name="sb_small", bufs=2))
    tp = ctx.enter_context(tc.tile_pool(name="tp", bufs=1))
    ps_big = ctx.enter_context(tc.tile_pool(name="ps_big", bufs=4, space="PSUM"))
    ps_acc = ctx.enter_context(tc.tile_pool(name="ps_acc", bufs=2, space="PSUM"))
    ps_sm = ctx.enter_context(tc.tile_pool(name="ps_sm", bufs=2, space="PSUM"))

    dma = nc.sync.dma_start

    def mm(out, lhsT, rhs, start=True, stop=True):
        nc.tensor.matmul(out, lhsT, rhs, start=start, stop=stop)

    # ---------------- per-core constants ----------------
    ident = sg.tile([128, 128], F32, tag="ident")
    dma(out=ident[:], in_=din["c_ident"][:, :])
    ident_r = sg.tile([128, 128], F32R, tag="ident_r")
    nc.vector.tensor_copy(ident_r[:], ident[:])
    ones_f = sg.tile([128, 128], F32, tag="ones_f")
    dma(out=ones_f[:], in_=din["c_ones"][:, :])
    ones128 = sg.tile([128, 128], F32R, tag="ones128")
    nc.vector.tensor_copy(ones128[:], ones_f[:])
    ones_inv = sg.tile([128, 128], F32R, tag="ones_inv")
    onesinv_s = sb_act.tile([128, 128], F32, tag="wstage", name="ws_onesinv")
    dma(out=onesinv_s[:], in_=din["c_onesinv"][:, :])
    nc.vector.tensor_copy(ones_inv[:], onesinv_s[:])
    eps_c = sg.tile([128, 1], F32, tag="eps_c")
    dma(out=eps_c[:], in_=din["c_eps"][:, :])

    def load_w(name, kchunks, cols, stage_tag="wstage"):
        stage = sb_act.tile([128, kchunks, cols], F32, tag=stage_tag,
                            name=f"ws_{name}")
        for c in range(kchunks):
            dma(out=stage[:, c, :], in_=din[name][c * 128:(c + 1) * 128, :])
        return stage

    def conv_w(name, stage, scale_col=None, scale_const=None):
        kchunks, cols = stage.shape[1], stage.shape[2]
        t = sg.tile([128, kchunks, cols], F32R, tag=f"w_{name}")
        for c in range(kchunks):
            if scale_col is not None:
                nc.vector.tensor_scalar_mul(
                    t[:, c, :], stage[:, c, :], scale_col[:, c:c + 1])
            elif scale_const is not None:
                nc.vector.tensor_scalar(
                    t[:, c, :], stage[:, c, :], scale_const, None, ALU.mult)
            else:
                nc.vector.tensor_copy(t[:, c, :], stage[:, c, :])
        return t

    def load_col(name, chunks):
        t = sg.tile([128, chunks], F32, tag=f"c_{name}")
        for c in range(chunks):
            dma(out=t[:, c:c + 1], in_=din[name][c * 128:(c + 1) * 128][:, None])
        return t

    bg_c = load_col("bg", 3)
    bl_c = load_col("bl", 3)
    lng1 = load_col("ln1_g", 3)
    lnb1 = load_col("ln1_b", 3)
    lng2 = load_col("ln2_g", 3)
    lnb2 = load_col("ln2_b", 3)
    bo_c = load_col("bo", 3)
    b1_c = load_col("b1", 1)
    b2_c = load_col("b2", 3)

    wg = conv_w("Wg", load_w("Wg", 4, DIM))
    wl = conv_w("Wl", load_w("Wl", 2, DIM))
    w2 = conv_w("W2", load_w("W2", 1, DIM))
    wk = conv_w("Wk", load_w("Wk", 3, DIM))
    wq = conv_w("Wq", load_w("Wq", 3, DIM), scale_const=DH ** -0.5)
    wo_s = sb_act.tile([TH * THD, DIM], F32, tag="wstage", name="ws_Wo")
    dma(out=wo_s[:], in_=din["Wo"][:, :])
    wo = sg.tile([TH * THD, DIM], F32R, tag="w_Wo")
    nc.vector.tensor_copy(wo[:], wo_s[:])

    # qkv/ff1 weights: bias fold (from raw weights, fp32 matmuls with N=1)
    # then g-scaled fp32r conversion.
    wqkv_s = load_w("Wqkv", 3, 288, stage_tag="wstage_qkv")
    w1_s = load_w("W1", 3, 128, stage_tag="wstage_w1")

    def fold_bias_col(tagn, lnb_col, stage, kchunks, cols, gw):
        ng = cols // gw
        col = sg.tile([128, ng], F32, tag=f"bc_{tagn}")
        for m in range(ng):
            ps2 = ps_sm.tile([gw, 1], F32, tag="pss")
            for c in range(kchunks):
                nc.tensor.matmul(
                    ps2[:, :], stage[:, c, m * gw:(m + 1) * gw],
                    lnb_col[:, c:c + 1],
                    start=(c == 0), stop=(c == kchunks - 1))
            nc.scalar.copy(col[:gw, m:m + 1], ps2[:, :])
        return col

    biasq_c = fold_bias_col("bq", lnb1, wqkv_s, 3, 288, 96)
    biasf_c = fold_bias_col("bf1", lnb2, w1_s, 3, 128, 128)
    bff_c = sg.tile([128, 1], F32, tag="bff")
    nc.vector.tensor_add(bff_c[:], biasf_c[:, 0:1], b1_c[:, 0:1])

    wqkv = conv_w("Wqkv", wqkv_s, scale_col=lng1)
    w1 = conv_w("W1", w1_s, scale_col=lng2)

    # att_bias broadcast column [128, HOUT] + per-partition sum (fp32 K=1).
    ab_row = sb_small.tile([1, HOUT], F32, tag="ab_row")
    dma(out=ab_row[:], in_=din["att_bias"].ap().rearrange("a h c d -> a (h c d)"))
    ps_ab = ps_sm.tile([128, HOUT], F32, tag="pss")
    nc.tensor.matmul(ps_ab[:, :], ones_f[:1, :], ab_row[:])
    ab_bc = sg.tile([128, HOUT], F32, tag="ab_bc")
    nc.scalar.copy(ab_bc[:], ps_ab[:])
    absum_c = sg.tile([128, 1], F32, tag="absum")
    nc.vector.tensor_reduce(absum_c[:], ab_bc[:], AX.X, ALU.add)

    def load_row(name, n):
        t = sg.tile([1, n], F32, tag=f"r_{name}")
        dma(out=t[:], in_=din[name][:][None, :])
        return t

    bn_g_r = load_row("bn_g", HID)
    bn_b_r = load_row("bn_b", HID)
    bn_m_r = load_row("bn_mean", HID)
    bn_v_r = load_row("bn_var", HID)
    wf_r = sg.tile([1, HID], F32, tag="wf_r")
    dma(out=wf_r[:], in_=din["Wf"].ap().rearrange("a b -> b a"))
    bf_r = load_row("bf", 1)

    bn_s_r = sg.tile([1, HID], F32, tag="bn_s")
    nc.scalar.activation(bn_s_r[:], bn_v_r[:], AF.Sqrt, bias=eps_c[0:1, :])
    nc.vector.reciprocal(bn_s_r[:], bn_s_r[:])
    nc.vector.tensor_mul(bn_s_r[:], bn_s_r[:], bn_g_r[:])
    bn_o_r = sg.tile([1, HID], F32, tag="bn_o")
    nc.vector.tensor_mul(bn_o_r[:], bn_m_r[:], bn_s_r[:])
    nc.vector.tensor_sub(bn_o_r[:], bn_b_r[:], bn_o_r[:])

    # ---------------- helpers ----------------
    def pe_transpose_to(dst, src_ap):
        """dst = src_ap.T via PE transpose-mode + DVE eviction."""
        p, f = src_ap.shape
        idt = ident_r if src_ap.dtype == F32R else ident
        ps = ps_sm.tile([128, 128], src_ap.dtype, tag="pss")
        nc.tensor.transpose(ps[:f, :p], src_ap, idt[:p, :p])
        nc.vector.tensor_copy(dst, ps[:f, :p])

    def layernorm(xt, mtag):
        del mtag
        xsq3 = tp.tile([128, 3, 512], F32R, tag="xsq3")
        for c in range(3):
            nc.scalar.activation(xsq3[:, c, :], xt[:, c, :], AF.Square)
        ps_mean = ps_big.tile([128, 512], F32, tag="ps")
        for c in range(3):
            mm(ps_mean[:, :], ones_inv[:], xt[:, c, :],
               start=(c == 0), stop=(c == 2))
        ps_ms = ps_big.tile([128, 512], F32, tag="ps")
        for c in range(3):
            mm(ps_ms[:, :], ones_inv[:], xsq3[:, c, :],
               start=(c == 0), stop=(c == 2))
        mean_bc = tp.tile([128, 512], F32, tag="mean")
        nc.scalar.copy(mean_bc[:], ps_mean[:])
        var_bc = tp.tile([128, 512], F32, tag="var")
        nc.vector.tensor_mul(var_bc[:], mean_bc[:], mean_bc[:])
        nc.vector.tensor_sub(var_bc[:], ps_ms[:], var_bc[:])
        nc.scalar.activation(var_bc[:], var_bc[:], AF.Sqrt, bias=eps_c[:, :])
        nc.vector.reciprocal(var_bc[:], var_bc[:])
        ht = tp.tile([128, 3, 512], F32R, tag="h")
        for c in range(3):
            nc.vector.tensor_sub(ht[:, c, :], xt[:, c, :], mean_bc[:])
            nc.vector.tensor_mul(ht[:, c, :], ht[:, c, :], var_bc[:])
        return ht

    def transformer(xt, otag):
        ht = layernorm(xt, "1")

        qkv_t = tp.tile([TH * THD, 3, 512], F32R, tag="qkv")
        for grp in range(3):
            ps = ps_big.tile([128, 512], F32, tag="ps")
            for c in range(3):
                mm(ps[:TH * THD, :], wqkv[:, c, grp * 96:(grp + 1) * 96],
                   ht[:, c, :], start=(c == 0), stop=(c == 2))
            nc.scalar.activation(
                qkv_t[:, grp, :], ps[:TH * THD, :], AF.Identity,
                bias=biasq_c[0:TH * THD, grp:grp + 1],
            )
        qt = qkv_t[:, 0, :]
        kt = qkv_t[:, 1, :]
        vt = qkv_t[:, 2, :]

        v_tok = tp.tile([128, 4, TH * THD], F32R, tag="vtok")
        for jt in range(4):
            pe_transpose_to(v_tok[:, jt, :], vt[:, jt * 128:(jt + 1) * 128])

        pts = []
        for h in range(TH):
            pt_h = tp.tile([128, 4, 512], F32R, tag=f"pt{h}", name=f"pt{h}")
            pts.append(pt_h)
        for h in range(TH):
            hr = slice(h * THD, (h + 1) * THD)
            for jt in range(4):
                psd = ps_big.tile([128, 512], F32, tag="ps", name=f"psd{h}_{jt}")
                mm(psd[:, :], kt[hr, jt * 128:(jt + 1) * 128], qt[hr, :])
                nc.scalar.activation(
                    pts[h][:, jt, :], psd[:, :], AF.Exp, scale=THD ** -0.5
                )
        recip3 = tp.tile([TH * THD, 512], F32, tag="recip3")
        for h in range(TH):
            hr = slice(h * THD, (h + 1) * THD)
            ps_s3 = ps_big.tile([THD, 512], F32, tag="ps", name=f"ps_s3_{h}")
            for jt in range(4):
                mm(ps_s3[:, :], ones128[:, :THD], pts[h][:, jt, :],
                   start=(jt == 0), stop=(jt == 3))
            nc.vector.reciprocal(recip3[hr, :], ps_s3[:, :])
        aot = tp.tile([TH * THD, 512], F32R, tag="aot")
        for h in range(TH):
            hr = slice(h * THD, (h + 1) * THD)
            ps_pv = ps_big.tile([THD, 512], F32, tag="ps", name=f"ps_pv_{h}")
            for jt in range(4):
                mm(ps_pv[:, :], v_tok[:, jt, hr], pts[h][:, jt, :],
                   start=(jt == 0), stop=(jt == 3))
            nc.vector.tensor_mul(aot[hr, :], ps_pv[:, :], recip3[hr, :])

        x2 = tp.tile([128, 3, 512], F32R, tag="x2")
        for mt in range(3):
            ps = ps_big.tile([128, 512], F32, tag="ps")
            mm(ps[:, :], wo[:, mt * 128:(mt + 1) * 128], aot[:, :])
            nc.vector.scalar_tensor_tensor(
                x2[:, mt, :], ps[:, :], bo_c[:, mt:mt + 1], xt[:, mt, :],
                ALU.add, ALU.add,
            )

        h2 = layernorm(x2, "2")
        g1 = tp.tile([128, 512], F32R, tag="g1")
        ps = ps_big.tile([128, 512], F32, tag="ps")
        for c in range(3):
            mm(ps[:, :], w1[:, c, :], h2[:, c, :], start=(c == 0), stop=(c == 2))
        nc.scalar.activation(g1[:], ps[:], AF.Gelu, bias=bff_c[:, 0:1])
        yt = tp.tile([128, 3, 512], F32R, tag=f"y{otag}")
        for mt in range(3):
            ps = ps_big.tile([128, 512], F32, tag="ps")
            mm(ps[:, :], w2[:, 0, mt * 128:(mt + 1) * 128], g1[:])
            nc.vector.scalar_tensor_tensor(
                yt[:, mt, :], ps[:, :], b2_c[:, mt:mt + 1], x2[:, mt, :],
                ALU.add, ALU.add,
            )
        return yt

    # ---------------- per-batch ----------------
    for b in range(BPC):
        g_tok = sb_in.tile([128, 4, GD], F32, tag="g_tok")
        for vt in range(4):
            dma(out=g_tok[:, vt, :], in_=din["g"][b:b + 1, vt * 128:(vt + 1) * 128, :].rearrange("a b c -> (a b) c"))
        l_tok = sb_in.tile([128, 4, LD], F32, tag="l_tok")
        for qt in range(4):
            dma(out=l_tok[:, qt, :], in_=din["l"][b:b + 1, qt * 128:(qt + 1) * 128, :].rearrange("a b c -> (a b) c"))

        # mask keep: 1.0 where sum|g| > 0 (per v row)
        asum = sb_small.tile([128, 4], F32, tag="asum")
        nc.vector.tensor_reduce(
            asum[:], g_tok[:, :, :], AX.X, ALU.add, apply_absolute_value=True
        )
        keep_s = sb_small.tile([128, 4], F32, tag="keep_s")
        nc.vector.tensor_scalar(keep_s[:], asum[:], 0.0, None, ALU.is_gt)
        ps_kr = ps_sm.tile([1, 512], F32, tag="pss")
        for vt in range(4):
            nc.tensor.transpose(ps_kr[:, vt * 128:(vt + 1) * 128],
                                keep_s[:, vt:vt + 1], ident[:])
        keep_row = sb_small.tile([1, 512], F32, tag="keep_row")
        nc.scalar.copy(keep_row[:], ps_kr[:])
        ps_kb = ps_big.tile([128, 512], F32, tag="ps")
        nc.tensor.matmul(ps_kb[:, :], ones_f[:1, :], keep_row[:])
        keep_bc = sb_tr.tile([128, 512], F32, tag="keep_bc")
        nc.scalar.copy(keep_bc[:], ps_kb[:])
        sbk_bc = sb_tr.tile([128, 512], F32, tag="sbk_bc")
        nc.vector.tensor_scalar_mul(sbk_bc[:], keep_bc[:], absum_c[:, 0:1])
        bk_c = sb_small.tile([128, HOUT, 4], F32, tag="bk_c")
        for h in range(HOUT):
            nc.vector.tensor_scalar_mul(
                bk_c[:, h, :], keep_s[:], ab_bc[:, h:h + 1]
            )

        # transpose inputs to feature-major
        gT = sb_in.tile([128, 4, 512], F32R, tag="gT")
        for i in range(4):
            for j in range(4):
                pe_transpose_to(
                    gT[:, i, j * 128:(j + 1) * 128],
                    g_tok[:, j, i * 128:(i + 1) * 128],
                )
        lT = sb_in.tile([128, 2, 512], F32R, tag="lT")
        for i in range(2):
            for j in range(4):
                pe_transpose_to(
                    lT[:, i, j * 128:(j + 1) * 128],
                    l_tok[:, j, i * 128:(i + 1) * 128],
                )

        # input projections + relu
        gp = sb_act.tile([128, 3, 512], F32R, tag="gp")
        for mt in range(3):
            ps = ps_big.tile([128, 512], F32, tag="ps")
            for c in range(4):
                mm(ps[:, :], wg[:, c, mt * 128:(mt + 1) * 128], gT[:, c, :],
                   start=(c == 0), stop=(c == 3))
            nc.scalar.activation(gp[:, mt, :], ps[:, :], AF.Relu,
                                 bias=bg_c[:, mt:mt + 1])
        lp = sb_act.tile([128, 3, 512], F32R, tag="lp")
        for mt in range(3):
            ps = ps_big.tile([128, 512], F32, tag="ps")
            for c in range(2):
                mm(ps[:, :], wl[:, c, mt * 128:(mt + 1) * 128], lT[:, c, :],
                   start=(c == 0), stop=(c == 1))
            nc.scalar.activation(lp[:, mt, :], ps[:, :], AF.Relu,
                                 bias=bl_c[:, mt:mt + 1])

        ga = transformer(gp, "ga")
        la = transformer(lp, "la")

        # q/k head projections (feature-major, head-major storage)
        qh = sb_in.tile([DH, HOUT, 512], F32R, tag="g_tok")
        kh = sb_in.tile([DH, HOUT, 512], F32R, tag="gT")
        for h in range(HOUT):
            psq = ps_big.tile([128, 512], F32, tag="ps")
            for c in range(3):
                mm(psq[:DH, :], wq[:, c, h * DH:(h + 1) * DH], ga[:, c, :],
                   start=(c == 0), stop=(c == 2))
            nc.scalar.copy(qh[:, h, :], psq[:DH, :])
            psk = ps_big.tile([128, 512], F32, tag="ps")
            for c in range(3):
                mm(psk[:DH, :], wk[:, c, h * DH:(h + 1) * DH], la[:, c, :],
                   start=(c == 0), stop=(c == 2))
            nc.scalar.copy(kh[:, h, :], psk[:DH, :])

        # scores out (token-major), mask+bias folded into ACT eviction
        for h in range(HOUT):
            for vt in range(4):
                ps = ps_big.tile([128, 512], F32, tag="ps")
                mm(ps[:, :], qh[:, h, vt * 128:(vt + 1) * 128], kh[:, h, :])
                sc = sb_stg.tile([128, 512], F32, tag="sc")
                nc.scalar.activation(
                    sc[:], ps[:], AF.Identity,
                    bias=bk_c[:, h, vt:vt + 1], scale=keep_s[:, vt:vt + 1],
                )
                dma(out=scores_out[b:b + 1, h:h + 1, vt * 128:(vt + 1) * 128, :].rearrange("a b c d -> (a b c) d"), in_=sc[:])

        # S^T = sum_h scores_h^T with mask+bias at eviction
        st = sb_in.tile([128, 4, 512], F32R, tag="l_tok")
        for qt in range(4):
            ps = ps_acc.tile([128, 512], F32, tag="ps")
            for h in range(HOUT):
                mm(ps[:, :], kh[:, h, qt * 128:(qt + 1) * 128], qh[:, h, :],
                   start=(h == 0), stop=(h == HOUT - 1))
            nc.vector.tensor_mul(st[:, qt, :], ps[:, :], keep_bc[:])
            nc.vector.tensor_add(st[:, qt, :], st[:, qt, :], sbk_bc[:])

        # la token-major
        latok = sb_in.tile([128, 4, DIM], F32R, tag="lT")
        for qt in range(4):
            for dt in range(3):
                pe_transpose_to(
                    latok[:, qt, dt * 128:(dt + 1) * 128],
                    la[:, dt, qt * 128:(qt + 1) * 128],
                )

        # T^T = la_tok.T @ S^T, then logits[d] = sum_v ga.T * T^T
        logits3 = sb_small.tile([128, 3], F32, tag="logits3")
        for dt in range(3):
            ps = ps_acc.tile([128, 512], F32, tag="ps")
            for qt in range(4):
                mm(ps[:, :], latok[:, qt, dt * 128:(dt + 1) * 128],
                   st[:, qt, :], start=(qt == 0), stop=(qt == 3))
            scratch = sb_act.tile([128, 512], F32, tag="scratch")
            nc.vector.tensor_mul(scratch[:], ga[:, dt, :], ps[:, :])
            nc.vector.tensor_reduce(
                logits3[:, dt:dt + 1], scratch[:], AX.X, ALU.add)

        # pooled(3): lrow[d] = logits[d], then sum consecutive triples
        ps_l = ps_sm.tile([1, 512], F32, tag="pss")
        for dt in range(3):
            nc.tensor.transpose(ps_l[:, dt * 128:(dt + 1) * 128],
                                logits3[:, dt:dt + 1], ident[:])
        lrow = sb_small.tile([1, DIM], F32, tag="lrow")
        nc.vector.tensor_copy(lrow[:], ps_l[:, :DIM])
        r3 = lrow.rearrange("a (j three) -> a three j", three=3)
        prow = sb_small.tile([1, HID], F32, tag="prow")
        nc.vector.tensor_add(prow[:], r3[:, 0, :], r3[:, 1, :])
        nc.vector.tensor_add(prow[:], prow[:], r3[:, 2, :])
        nc.vector.tensor_mul(prow[:], prow[:], bn_s_r[:])
        nc.vector.tensor_add(prow[:], prow[:], bn_o_r[:])
        o11 = sb_small.tile([1, 1], F32, tag="o11")
        oscr = sb_small.tile([1, HID], F32, tag="oscr")
        nc.vector.tensor_mul(oscr[:], prow[:], wf_r[:])
        nc.vector.tensor_reduce(o11[:], oscr[:], AX.X, ALU.add)
        nc.vector.tensor_add(o11[:], o11[:], bf_r[:])
        dma(out=out_out[b:b + 1, :], in_=o11[:])


# ---------------------------------------------------------------- host side
_CACHE = {}


def _get_nc():
    if "nc" not in _CACHE:
        _CACHE["nc"] = build_bass()
    return _CACHE["nc"]


def kernel(**inputs):
    inputs = {k: np.ascontiguousarray(np.asarray(v), dtype=None)
              for k, v in inputs.items()}
    nc = _get_nc()

    from concourse.bass_utils import run_bass_kernel_spmd

    consts = {
        "c_ident": np.eye(128, dtype=np.float32),
        "c_ones": np.ones((128, 128), np.float32),
        "c_onesinv": np.full((128, 128), 1.0 / DIM, np.float32),
        "c_eps": np.full((128, 1), EPS, np.float32),
    }
    in_maps = []
    for c in range(N_CORES):
        m = {k: np.asarray(v) for k, v in inputs.items()
             if k not in ("global_feat", "local_feat")}
        m.update(consts)
        m["g"] = np.ascontiguousarray(inputs["global_feat"][c * BPC:(c + 1) * BPC])
        m["l"] = np.ascontiguousarray(inputs["local_feat"][c * BPC:(c + 1) * BPC])
        in_maps.append(m)

    res = run_bass_kernel_spmd(nc, in_maps, list(range(N_CORES)))
    out = np.concatenate([res.results[c]["out"] for c in range(N_CORES)], axis=0)
    scores = np.concatenate(
        [res.results[c]["scores"] for c in range(N_CORES)], axis=0
    )
    return out, scores
